# revision 31
# baseline (speedup 1.0000x reference)
"""Trainium2 Bass kernel for BalancedConformationalConsistencyLoss.

Strategy (segment/fragment parallelism, 8 cores):
  * Host sorts nodes by fragment; 32 fragments per core (snake-deal by size),
    bin-packed into 9 strips of 128 slots so no fragment straddles a strip.
  * Device (per core, SPMD), all-bf16 matmul path:
      - encoder MLPs feature-major, softplus split into Exp/Ln phases batched
        by activation function (5 table loads total);
      - per-node squared norms via squared-features + per-strip one-column
        matmuls (node-major), inverse norms via Ln/Exp(-0.5);
      - norm rows bounced through DRAM to node-row layout, broadcast across
        partitions on GpSimd, features normalized on DVE;
      - per-strip 128x128 gram blocks (normalized + unnormalized) with masked
        pair reductions via scalar_tensor_tensor / activation accumulators;
      - per-fragment sums via one-hot matmuls; per-core partials -> [1,3].
  * Host combines 8 partials (sum + the two global Frobenius sqrts).

All pairwise math uses the identity  sum_pm (S-t)^2 = A - 2tB + t^2*npairs
with A = sum pm*S^2, B = sum pm*S, so no (S-t) intermediates are built.
"""
import numpy as np
import ml_dtypes
from contextlib import ExitStack

BF = ml_dtypes.bfloat16

# ---------------- problem constants (hardcoded per contract) ----------------
N, D, NF = 8192, 256, 256
R = 0.3
BRICS = 0.4
AW = 0.6
VW = 0.2
CF = 0.1
NCORES = 8
NFC = NF // NCORES          # 32 fragments per core
P = 128                     # strip height
NSTRIP = 9                  # strips per core
M = NSTRIP * P              # 1152 padded slots per core
CH = 384                    # encoder column chunk
NCH = M // CH
LN2 = np.float32(np.log(2.0))

# Q column indices (per-strip accumulator tile)
(QB_SH, QA_SH, QA_UN, QC_UN, QB_DIR, QA_DIR, QSOFF, QVOFF, QJK) = range(9)
NQ = 9
# cvec columns: 0..8 coeffs for Q cols, 9..13 weights for segn (nsq per set),
# 14 = additive const
NCV = 15

# weight-pack column offsets in the [128, 6144] packed weight tensor
_WOFF = {}
_off = 0
for _nm, _kt, _cols in [("w1c", 4, D), ("w2", 2, D), ("wd1", 2, 2 * D),
                        ("wd2", 4, 2 * D), ("wv1c", 4, D), ("wv2", 2, D)]:
    _WOFF[_nm] = (_off, _kt, _cols)
    _off += _kt * _cols
WCOLS = _off  # 6144


# ============================ host-side prep ================================

def _assign_fragments(fid):
    counts = np.bincount(fid, minlength=NF)
    order = np.argsort(-counts, kind="stable")
    core_frags = [[] for _ in range(NCORES)]
    for i, f in enumerate(order):
        r = i // NCORES
        c = i % NCORES if r % 2 == 0 else NCORES - 1 - (i % NCORES)
        core_frags[c].append(int(f))
    layout = []
    for c in range(NCORES):
        frags = sorted(core_frags[c], key=lambda f: -counts[f])
        strips = [[] for _ in range(NSTRIP)]
        fill = np.zeros(NSTRIP, dtype=int)
        for f in frags:
            for s in range(NSTRIP):
                if fill[s] + counts[f] <= P:
                    strips[s].append(f)
                    fill[s] += counts[f]
                    break
            else:
                raise AssertionError(f"core {c}: fragment {f} does not fit")
        layout.append(strips)
    return layout, counts


def _build_core_meta(fid, atom_types, layout, counts):
    nodes_of = {f: np.nonzero(fid == f)[0] for f in range(NF)}
    metas = []
    for c in range(NCORES):
        slot_node = -np.ones(M, dtype=np.int64)
        slot_frag = -np.ones(M, dtype=np.int64)
        frag_global = []
        fl = 0
        for b in range(NSTRIP):
            pos = b * P
            for f in layout[c][b]:
                nn = nodes_of[f]
                slot_node[pos:pos + len(nn)] = nn
                slot_frag[pos:pos + len(nn)] = fl
                frag_global.append(f)
                pos += len(nn)
                fl += 1
        assert fl == NFC
        frag_global = np.array(frag_global, dtype=np.int64)
        real = slot_node >= 0

        pm3 = np.zeros((NSTRIP, P, 3 * P), dtype=np.float32)
        tm = np.zeros((NSTRIP, P, P), dtype=np.float32)
        amat = np.zeros((NSTRIP, P, NFC), dtype=np.float32)
        t2pm_frag = np.zeros(NFC, dtype=np.float32)
        for b in range(NSTRIP):
            sf = slot_frag[b * P:(b + 1) * P]
            sn = slot_node[b * P:(b + 1) * P]
            rr = sf >= 0
            same = (sf[:, None] == sf[None, :]) & rr[:, None] & rr[None, :]
            upper = np.triu(np.ones((P, P), dtype=bool), k=1)
            pmb = (same & upper).astype(np.float32)
            pm3[b] = np.tile(pmb, (1, 3))
            at = np.where(rr, atom_types[np.where(rr, sn, 0)], -1)
            tgt = np.where(at[:, None] == at[None, :], 0.3, 0.1).astype(np.float32)
            tm[b] = tgt * pmb
            for p in range(P):
                if rr[p]:
                    amat[b, p, sf[p]] = 1.0
            for fl_ in np.unique(sf[rr]):
                sel = sf == fl_
                t2pm_frag[fl_] += float(((tgt * pmb) ** 2)[np.ix_(sel, sel)].sum())

        cnt = counts[frag_global].astype(np.float32)
        valid = (cnt >= 2.0).astype(np.float32)
        pairs = cnt * (cnt - 1.0) * 0.5
        safe_c = np.maximum(cnt, 1.0)
        safe_p = np.maximum(pairs, 1.0)

        t_sh = np.float32(BRICS)
        t_dir = np.float32(BRICS - 0.2)
        cvec = np.zeros((NFC, NCV), dtype=np.float32)
        cvec[:, QB_SH] = -2.0 * t_sh * AW / safe_p
        cvec[:, QA_SH] = AW / safe_p
        cvec[:, QA_UN] = (1.0 - AW) / safe_p
        cvec[:, QC_UN] = -2.0 * (1.0 - AW) / safe_p
        cvec[:, QB_DIR] = -2.0 * t_dir * VW / (3.0 * safe_p)
        cvec[:, QA_DIR] = VW / (3.0 * safe_p)
        cvec[:, QSOFF] = -2.0 * AW / (safe_c * safe_c)
        cvec[:, QVOFF] = -2.0 * VW / (safe_c * safe_c)
        cvec[:, QJK] = 0.0
        cvec[:, 9] = AW * (1.0 / safe_c - 1.0 / (safe_c * safe_c))    # nsq_sh
        cvec[:, 10] = 0.0                                             # nsq_un
        wv_ = VW * (1.0 / safe_c - 1.0 / (safe_c * safe_c))
        cvec[:, 11] = wv_
        cvec[:, 12] = wv_
        cvec[:, 13] = wv_
        cvec[:, 14] = (AW * t_sh * t_sh * pairs / safe_p
                       + (1.0 - AW) * t2pm_frag / safe_p
                       + VW * t_dir * t_dir * pairs / safe_p)
        cvec *= valid[:, None]

        metas.append(dict(slot_node=slot_node, real=real, pm3=pm3, tm=tm,
                          amat=amat, cvec=cvec))
    return metas


def _pack_rows(x, kt):
    # [kt*128, cols] -> [128, kt*cols] so each partition's data is contiguous
    cols = x.shape[1]
    return np.ascontiguousarray(
        x.reshape(kt, P, cols).transpose(1, 0, 2).reshape(P, kt * cols))


def _shard_inputs(inputs):
    fid = np.asarray(inputs["fragment_ids"]).astype(np.int64)
    at = np.asarray(inputs["atom_types"]).astype(np.int64)
    layout, counts = _assign_fragments(fid)
    metas = _build_core_meta(fid, at, layout, counts)

    W1 = np.asarray(inputs["W1"], np.float32)
    W2 = np.asarray(inputs["W2"], np.float32)
    Wd1 = np.asarray(inputs["Wd1"], np.float32)
    Wd2 = np.asarray(inputs["Wd2"], np.float32)
    Wv1 = np.asarray(inputs["Wv1"], np.float32)
    Wv2 = np.asarray(inputs["Wv2"], np.float32)
    w1c = np.concatenate([R * W1, (1.0 - R) * W1], axis=0)
    wv1c = np.concatenate([R * Wv1, (1.0 - R) * Wv1], axis=0)
    wall = np.concatenate([
        _pack_rows(w1c, 4), _pack_rows(W2, 2), _pack_rows(Wd1, 2),
        _pack_rows(Wd2, 4), _pack_rows(wv1c, 4), _pack_rows(Wv2, 2)],
        axis=1).astype(BF)
    assert wall.shape == (P, WCOLS)

    b2p = (np.asarray(inputs["b2"], np.float32) - LN2 * W2.sum(axis=0))
    bd2p = (np.asarray(inputs["bd2"], np.float32) - LN2 * Wd2.sum(axis=0))
    bv2p = (-LN2 * Wv2.sum(axis=0))
    ball = np.concatenate([
        np.asarray(inputs["b1"], np.float32).reshape(2, P).T,
        b2p.reshape(2, P).T,
        np.asarray(inputs["bd1"], np.float32).reshape(4, P).T,
        bd2p.reshape(4, P).T,
        bv2p.reshape(2, P).T], axis=1).astype(np.float32)
    assert ball.shape == (P, 14)

    ss = np.asarray(inputs["scalar_short"], np.float32)
    sl = np.asarray(inputs["scalar_long"], np.float32)
    vs = np.asarray(inputs["vector_short"], np.float32)
    vl = np.asarray(inputs["vector_long"], np.float32)

    in_maps = []
    for c in range(NCORES):
        m = metas[c]
        idx = np.where(m["real"], m["slot_node"], 0)

        def take2(x):
            g = (x[idx] * m["real"][:, None]).astype(np.float32)
            return _pack_rows(np.ascontiguousarray(g.T), 2).astype(BF)

        def take3(x):
            g = (x[idx] * m["real"][:, None, None]).astype(np.float32)
            t = np.ascontiguousarray(g.transpose(1, 2, 0).reshape(3 * D, M))
            return _pack_rows(t, 6).astype(BF)

        in_maps.append({
            "x_s": take2(ss), "x_l": take2(sl),
            "v_s": take3(vs), "v_l": take3(vl),
            "wall": wall, "ball": ball,
            "pm3": np.ascontiguousarray(
                m["pm3"].transpose(1, 0, 2).reshape(P, NSTRIP * 3 * P)).astype(BF),
            "tm": np.ascontiguousarray(
                m["tm"].transpose(1, 0, 2).reshape(P, NSTRIP * P)).astype(BF),
            "amat": np.ascontiguousarray(
                m["amat"].transpose(1, 0, 2).reshape(P, NSTRIP * NFC)),
            "cvec": m["cvec"],
            "ident": np.ascontiguousarray(np.eye(P, dtype=np.float32)).astype(BF),
        })
    n_valid = float((counts >= 2).sum())
    return in_maps, n_valid


def _combine(fins, n_valid):
    loss = float(sum(float(f[0]) for f in fins))
    ssq_sh = float(sum(float(f[1]) for f in fins))
    ssq_un = float(sum(float(f[2]) for f in fins))
    l2 = 0.01 * (np.sqrt(ssq_sh) + np.sqrt(ssq_un))
    if n_valid > 0:
        return np.float32(CF * (loss + n_valid * l2) / max(n_valid, 1.0))
    return np.float32(0.0)


# ============================ device program ================================

_NC_CACHE = {}


def build_nc(debug_out=False):
    key = ("nc", debug_out)
    if key in _NC_CACHE:
        return _NC_CACHE[key]
    import concourse.bass as bass
    import concourse.bacc as bacc
    import concourse.mybir as mybir
    import concourse.tile as tile

    F32 = mybir.dt.float32
    BF16 = mybir.dt.bfloat16
    AF = mybir.ActivationFunctionType
    ALU = mybir.AluOpType

    nc = bacc.Bacc("TRN2", target_bir_lowering=False, debug=False)

    d = {}
    for name, shape, dt_ in [
        ("x_s", [P, 2 * M], BF16), ("x_l", [P, 2 * M], BF16),
        ("v_s", [P, 6 * M], BF16), ("v_l", [P, 6 * M], BF16),
        ("wall", [P, WCOLS], BF16), ("ball", [P, 14], F32),
        ("pm3", [P, NSTRIP * 3 * P], BF16), ("tm", [P, NSTRIP * P], BF16),
        ("amat", [P, NSTRIP * NFC], F32), ("cvec", [NFC, NCV], F32),
        ("ident", [P, P], BF16),
    ]:
        d[name] = nc.dram_tensor(name, shape, dt_, kind="ExternalInput").ap()
    d_out = nc.dram_tensor("out", [1, 3], F32, kind="ExternalOutput").ap()
    d_inv = nc.dram_tensor("inv_scratch", [NCH * 15, P], BF16).ap()
    d_dbg = {}
    if debug_out:
        for nm, shape, dt_ in [("dbg_nsq", [P, 45], F32),
                               ("dbg_seg", [NFC, NQ], F32),
                               ("dbg_segn", [NFC, 5], F32),
                               ("dbg_irow", [1, 5 * M], F32)]:
            d_dbg[nm] = nc.dram_tensor(nm, shape, dt_, kind="ExternalOutput").ap()

    with tile.TileContext(nc) as tc, ExitStack() as ctx:
        wpool = ctx.enter_context(tc.tile_pool(name="w", bufs=1))
        feat = ctx.enter_context(tc.tile_pool(name="feat", bufs=1))
        xin = ctx.enter_context(tc.tile_pool(name="xin", bufs=2))
        small = ctx.enter_context(tc.tile_pool(name="small", bufs=1))
        rowp = ctx.enter_context(tc.tile_pool(name="rowp", bufs=2))
        junkp = ctx.enter_context(tc.tile_pool(name="junk", bufs=3))
        psp = ctx.enter_context(tc.tile_pool(name="ps", bufs=4, space="PSUM"))
        pnsq = ctx.enter_context(tc.tile_pool(name="pnsq", bufs=1, space="PSUM"))
        pseg = ctx.enter_context(tc.tile_pool(name="pseg", bufs=1, space="PSUM"))

        # ---- constants / weights (one DMA each); encoder inputs issue first
        # on the sync queue, G-phase masks go on the scalar queue ----
        def ld(name, shape, dt_, pool=wpool, eng=None):
            t = pool.tile(shape, dt_, tag=name, name=name)
            (eng or nc.sync).dma_start(out=t, in_=d[name])
            return t

        wall_sb = ld("wall", [P, WCOLS], BF16)
        ball_sb = ld("ball", [P, 14], F32)
        xs_sb = ld("x_s", [P, 2 * M], BF16, pool=feat)
        xl_sb = ld("x_l", [P, 2 * M], BF16, pool=feat)
        vs_sb = ld("v_s", [P, 6 * M], BF16, pool=feat)
        vl_sb = ld("v_l", [P, 6 * M], BF16, pool=feat)
        pm3_sb = ld("pm3", [P, NSTRIP * 3 * P], BF16, eng=nc.scalar)
        tm_sb = ld("tm", [P, NSTRIP * P], BF16, eng=nc.scalar)
        amat_sb = ld("amat", [P, NSTRIP * NFC], F32, eng=nc.scalar)
        cvec_sb = ld("cvec", [NFC, NCV], F32, eng=nc.scalar)
        ident_sb = ld("ident", [P, P], BF16, eng=nc.scalar)

        def W(name, k):
            off, kt, cols = _WOFF[name]
            assert k < kt
            return wall_sb[:, off + k * cols: off + (k + 1) * cols]

        def bias(name, m_):
            offs = {"b1": 0, "b2p": 2, "bd1": 4, "bd2p": 8, "bv2p": 12}
            return ball_sb[:, offs[name] + m_: offs[name] + m_ + 1]

        ones_col = wpool.tile([P, 1], BF16, tag="ones_col", name="ones_col")
        nc.vector.memset(ones_col, 1.0)
        ones32 = wpool.tile([NFC, 1], F32, tag="ones32", name="ones32")
        nc.vector.memset(ones32, 1.0)
        eps_sb = wpool.tile([P, 1], F32, tag="eps", name="eps")
        nc.vector.memset(eps_sb, 1e-20)

        # ---- persistent per-chunk feature tiles ----
        # sets order: 0=sh 1=un 2=v0 3=v1 4=v2 ; each set has k-tiles 0,1
        u_t = [[[feat.tile([P, CH], BF16, tag=f"u{s}_{k}_{c}", name=f"u{s}_{k}_{c}")
                 for c in range(NCH)] for k in range(2)] for s in range(5)]
        n_t = [[[feat.tile([P, CH], BF16, tag=f"n{s}_{k}_{c}", name=f"n{s}_{k}_{c}")
                 for c in range(NCH)] for k in range(2)] for s in range(5)]

        nsq_ps = pnsq.tile([P, 45], F32, tag="nsq", name="nsq_ps")

        def mm_chunk(pt, w_name, rhs_list, m_):
            kt = len(rhs_list)
            for k in range(kt):
                nc.tensor.matmul(pt[:, 0:CH], W(w_name, k)[:, m_ * P:(m_ + 1) * P],
                                 rhs_list[k], start=(k == 0), stop=(k == kt - 1))

        def xsl(t, k, c):
            return t[:, k * M + c * CH: k * M + (c + 1) * CH]

        # ---- encoder phase A: h1 = softplus(x@W1c + b1), split Exp/Ln ----
        e_h1, t_h1, s_all = [], [], []
        for c in range(NCH):
            ew = xin.tile([P, 2 * CH], BF16, tag="eh1", name=f"eh1_{c}", bufs=2)
            for m_ in range(2):
                pt = psp.tile([P, 512], F32, tag="ps", name="ps")
                mm_chunk(pt, "w1c", [xsl(xs_sb, 0, c), xsl(xs_sb, 1, c),
                                     xsl(xl_sb, 0, c), xsl(xl_sb, 1, c)], m_)
                nc.scalar.activation(ew[:, m_ * CH:(m_ + 1) * CH], pt[:, 0:CH],
                                     AF.Exp, bias=bias("b1", m_))
            e_h1.append(ew)
        for c in range(NCH):
            tw = xin.tile([P, 2 * CH], BF16, tag="th1", name=f"th1_{c}", bufs=2)
            nc.scalar.activation(tw, e_h1[c], AF.Ln, bias=1.0)
            t_h1.append(tw)
        for c in range(NCH):
            s_ = []
            for m_ in range(2):
                pt = psp.tile([P, 512], F32, tag="ps", name="ps")
                mm_chunk(pt, "w2", [t_h1[c][:, 0:CH], t_h1[c][:, CH:2 * CH]], m_)
                t = xin.tile([P, CH], BF16, tag=f"s{m_}", name=f"s{m_}_{c}", bufs=2)
                nc.vector.tensor_scalar_add(t, pt[:, 0:CH], bias("b2p", m_))
                s_.append(t)
            s_all.append(s_)

        # ---- encoder phase B: hd + v1 exps, wide Ln, then dec/v2 evacs ----
        e_B, t_B = [], []
        for c in range(NCH):
            ew = xin.tile([P, 10 * CH], BF16, tag="eB", name=f"eB_{c}", bufs=2)
            for m_ in range(4):
                pt = psp.tile([P, 512], F32, tag="ps", name="ps")
                mm_chunk(pt, "wd1", s_all[c], m_)
                nc.scalar.activation(ew[:, m_ * CH:(m_ + 1) * CH], pt[:, 0:CH],
                                     AF.Exp, bias=bias("bd1", m_))
            for dd in range(3):
                rhs = [xsl(vs_sb, 2 * dd, c), xsl(vs_sb, 2 * dd + 1, c),
                       xsl(vl_sb, 2 * dd, c), xsl(vl_sb, 2 * dd + 1, c)]
                for m_ in range(2):
                    pt = psp.tile([P, 512], F32, tag="ps", name="ps")
                    mm_chunk(pt, "wv1c", rhs, m_)
                    j = 4 + 2 * dd + m_
                    nc.scalar.activation(ew[:, j * CH:(j + 1) * CH], pt[:, 0:CH],
                                         AF.Exp)
            e_B.append(ew)
        for c in range(NCH):
            tw = xin.tile([P, 10 * CH], BF16, tag="tB", name=f"tB_{c}", bufs=2)
            nc.scalar.activation(tw, e_B[c], AF.Ln, bias=1.0)
            t_B.append(tw)

        xsq_t = [[None] * 2 for _ in range(5)]

        def sq_and_nsq(s, k, c, src):
            # xsq = src^2; the one-column matmuls that reduce it to nsq are
            # emitted after the chunk's evacs so each column's two-matmul
            # accumulation group is contiguous
            t = feat.tile([P, CH], BF16, tag=f"xsq{s}_{k}", name=f"xsq{s}_{k}")
            if s < 2:
                nc.gpsimd.tensor_mul(t, src, src)
            else:
                nc.vector.tensor_mul(t, src, src)
            xsq_t[s][k] = t

        def nsq_mms(c):
            for s in range(5):
                for ls in range(3):
                    col = s * NSTRIP + 3 * c + ls
                    for k in range(2):
                        nc.tensor.matmul(
                            nsq_ps[:, col:col + 1],
                            xsq_t[s][k][:, ls * P:(ls + 1) * P], ones_col,
                            start=(k == 0), stop=(k == 1))

        def msum(out_t, in0, in1, acc):
            nc.vector.scalar_tensor_tensor(
                out=out_t, in0=in0, scalar=1.0, in1=in1,
                op0=ALU.bypass, op1=ALU.mult, accum_out=acc)

        Q_b = [small.tile([P, NQ], F32, tag=f"Q{b}", name=f"Q{b}")
               for b in range(NSTRIP)]
        d_inv_rd = d_inv.rearrange("(o c r) p -> c o (r p)", o=1, c=NCH, r=15)

        def strip_work(b):
            cb, ls = b // 3, (b % 3)
            lsl = slice(ls * P, (ls + 1) * P)
            pmb = pm3_sb[:, b * 3 * P: b * 3 * P + P]
            pm3b = pm3_sb[:, b * 3 * P: (b + 1) * 3 * P]
            tmb = tm_sb[:, b * P:(b + 1) * P]
            Q = Q_b[b]

            def gpair(pt, col0, tiles):
                for k in range(2):
                    nc.tensor.matmul(pt[:, col0:col0 + P], tiles[k][cb][:, lsl],
                                     tiles[k][cb][:, lsl],
                                     start=(k == 0), stop=(k == 1))

            # unnormalized grams (sh + 3 v dims) packed in one bank
            gu = psp.tile([P, 512], F32, tag="ps", name="gu")
            gpair(gu, 0, u_t[0])
            for dd in range(3):
                gpair(gu, P + dd * P, u_t[2 + dd])
            j5 = junkp.tile([P, P], BF16, tag="jk1", name="j5")
            msum(j5, gu[:, 0:P], pmb, Q[:, QSOFF:QSOFF + 1])
            j6 = junkp.tile([P, 3 * P], BF16, tag="jk3", name="j6")
            msum(j6, gu[:, P:4 * P], pm3b, Q[:, QVOFF:QVOFF + 1])

            # normalized grams: [sh | un] bank and [v0 | v1 | v2] bank
            gn = psp.tile([P, 512], F32, tag="ps", name="gn")
            gpair(gn, 0, n_t[0])
            gpair(gn, P, n_t[1])
            gv = psp.tile([P, 512], F32, tag="ps", name="gv")
            for dd in range(3):
                gpair(gv, dd * P, n_t[2 + dd])

            spm = junkp.tile([P, P], BF16, tag="spm", name="spm", bufs=2)
            msum(spm, gn[:, 0:P], pmb, Q[:, QB_SH:QB_SH + 1])
            nc.scalar.activation(junkp.tile([P, P], BF16, tag="jk1", name="j1"),
                                 spm, AF.Square, accum_out=Q[:, QA_SH:QA_SH + 1])
            upm = junkp.tile([P, P], BF16, tag="upm", name="upm", bufs=2)
            msum(upm, gn[:, P:2 * P], pmb, Q[:, QJK:QJK + 1])
            nc.scalar.activation(junkp.tile([P, P], BF16, tag="jk1", name="j2"),
                                 upm, AF.Square, accum_out=Q[:, QA_UN:QA_UN + 1])
            j3 = junkp.tile([P, P], BF16, tag="jk1", name="j3")
            msum(j3, gn[:, P:2 * P], tmb, Q[:, QC_UN:QC_UN + 1])

            dpm = junkp.tile([P, 3 * P], BF16, tag="dpm", name="dpm", bufs=2)
            msum(dpm, gv[:, 0:3 * P], pm3b, Q[:, QB_DIR:QB_DIR + 1])
            nc.scalar.activation(junkp.tile([P, 3 * P], BF16, tag="jk3", name="j4"),
                                 dpm, AF.Square, accum_out=Q[:, QA_DIR:QA_DIR + 1])

        # per chunk: dec/v2 evacs -> nsq -> inv -> bounce -> bcast ->
        # normalize -> strip grams/reductions (pipelines across chunks)
        for c in range(NCH):
            hd = [t_B[c][:, m_ * CH:(m_ + 1) * CH] for m_ in range(4)]
            for m_ in range(4):
                pt = psp.tile([P, 512], F32, tag="ps", name="ps")
                mm_chunk(pt, "wd2", hd, m_)
                s, k = (0, m_) if m_ < 2 else (1, m_ - 2)
                nc.vector.tensor_scalar_add(u_t[s][k][c], pt[:, 0:CH],
                                            bias("bd2p", m_))
                sq_and_nsq(s, k, c, u_t[s][k][c])
            for dd in range(3):
                v1 = [t_B[c][:, (4 + 2 * dd + k) * CH:(5 + 2 * dd + k) * CH]
                      for k in range(2)]
                for m_ in range(2):
                    pt = psp.tile([P, 512], F32, tag="ps", name="ps")
                    mm_chunk(pt, "wv2", v1, m_)
                    nc.vector.tensor_scalar_add(u_t[2 + dd][m_][c], pt[:, 0:CH],
                                                bias("bv2p", m_))
                    sq_and_nsq(2 + dd, m_, c, u_t[2 + dd][m_][c])
            nsq_mms(c)

            # inv = (nsq+eps)^(-1/2) for this chunk's 15 (set, strip) columns
            nsq_c = nsq_ps.rearrange("p (s b) -> p s b", s=5)[:, :, 3 * c:3 * c + 3]
            lnc = junkp.tile([P, 15], F32, tag="lnc", name=f"lnc{c}", bufs=2)
            nc.scalar.activation(lnc.rearrange("p (s b) -> p s b", s=5), nsq_c,
                                 AF.Ln, bias=eps_sb[:, 0:1])
            invc = junkp.tile([P, 15], BF16, tag="invc", name=f"invc{c}", bufs=2)
            nc.scalar.activation(invc, lnc, AF.Exp, scale=-0.5)
            invT_ps = pseg.tile([15, P], BF16, tag="invT", name="invT")
            nc.tensor.transpose(invT_ps, invc, ident_sb)
            invT_c = junkp.tile([15, P], BF16, tag="invT_sb", name=f"invT{c}",
                                bufs=2)
            nc.vector.tensor_copy(invT_c, invT_ps)
            nc.sync.dma_start(out=d_inv[c * 15:(c + 1) * 15, :], in_=invT_c)
            irow_c = rowp.tile([1, 15 * P], BF16, tag="irow", name=f"irow{c}",
                               bufs=2)
            nc.sync.dma_start(out=irow_c, in_=d_inv_rd[c])
            for s in range(5):
                B_sc = rowp.tile([P, CH], BF16, tag="Bb", name=f"B{s}_{c}", bufs=3)
                nc.gpsimd.partition_broadcast(
                    B_sc, irow_c[0:1, s * CH:(s + 1) * CH])
                for k in range(2):
                    nc.vector.tensor_mul(n_t[s][k][c], u_t[s][k][c], B_sc)
            for ls in range(3):
                strip_work(3 * c + ls)

        nsq_sb = small.tile([P, 45], F32, tag="nsq_sb", name="nsq_sb")
        nc.vector.tensor_copy(nsq_sb, nsq_ps)

        # ---- segment reduction + final combine ----
        segq_ps = pseg.tile([NFC, NQ + 5], F32, tag="seg", name="seg")
        seg_ps = segq_ps[:, 0:NQ]
        segn_ps = segq_ps[:, NQ:NQ + 5]
        nsq_v = nsq_sb.rearrange("p (s b) -> p s b", s=5)
        for b in range(NSTRIP):
            nc.tensor.matmul(seg_ps, amat_sb[:, b * NFC:(b + 1) * NFC], Q_b[b],
                             start=(b == 0), stop=(b == NSTRIP - 1))
        for b in range(NSTRIP):
            nc.tensor.matmul(segn_ps, amat_sb[:, b * NFC:(b + 1) * NFC],
                             nsq_v[:, :, b],
                             start=(b == 0), stop=(b == NSTRIP - 1))
        segq = small.tile([NFC, NQ + 5], F32, tag="segs", name="segs")
        nc.vector.tensor_copy(segq, segq_ps)
        segs = segq[:, 0:NQ]
        segn = segq[:, NQ:NQ + 5]

        acc = small.tile([NFC, 3], F32, tag="acc", name="acc")
        junkq = small.tile([NFC, NQ + 5], F32, tag="junkq", name="junkq")
        # acc0 = sum_q cvec[q]*segq[q] over Q cols + segn cols, then + const
        nc.vector.scalar_tensor_tensor(
            out=junkq, in0=segq, scalar=1.0, in1=cvec_sb[:, 0:NQ + 5],
            op0=ALU.bypass, op1=ALU.mult, accum_out=acc[:, 0:1])
        nc.vector.tensor_add(acc[:, 0:1], acc[:, 0:1], cvec_sb[:, 14:15])
        nc.vector.tensor_copy(acc[:, 1:2], segn[:, 0:1])
        nc.vector.tensor_copy(acc[:, 2:3], segn[:, 1:2])

        if debug_out:
            nc.sync.dma_start(out=d_dbg["dbg_nsq"], in_=nsq_sb)
            nc.sync.dma_start(out=d_dbg["dbg_seg"], in_=segs)
            nc.sync.dma_start(out=d_dbg["dbg_segn"], in_=segn)
            irow_f = rowp.tile([1, 5 * M], F32, tag="irow_f", name="irow_f", bufs=1)
            nc.vector.tensor_copy(irow_f, irow)
            nc.sync.dma_start(out=d_dbg["dbg_irow"], in_=irow_f)

        fin_ps = pseg.tile([1, 3], F32, tag="fin", name="fin")
        nc.tensor.matmul(fin_ps, ones32, acc, start=True, stop=True)
        fin_sb = small.tile([1, 3], F32, tag="fin_sb", name="fin_sb")
        nc.vector.tensor_copy(fin_sb, fin_ps)
        nc.sync.dma_start(out=d_out, in_=fin_sb)

    nc.compile()
    _NC_CACHE[key] = nc
    return nc


# ============================== entry point =================================

def kernel(**inputs) -> np.ndarray:
    from concourse.bass_utils import run_bass_kernel_spmd

    in_maps, n_valid = _shard_inputs(inputs)
    nc = build_nc()
    res = run_bass_kernel_spmd(nc, in_maps, core_ids=list(range(NCORES)))
    fins = [r["out"].reshape(3) for r in res.results]
    return _combine(fins, n_valid)


# revision 36
# speedup vs baseline: 1.3699x; 1.3699x over previous
"""Trainium2 Bass kernel for BalancedConformationalConsistencyLoss.

Strategy (segment/fragment parallelism, 8 cores):
  * Host sorts nodes by fragment; 32 fragments per core (snake-deal by size),
    bin-packed into 9 strips of 128 slots so no fragment straddles a strip.
  * Device (per core, SPMD), all-bf16 matmul path:
      - encoder MLPs feature-major, softplus split into Exp/Ln phases batched
        by activation function (5 table loads total);
      - per-node squared norms via squared-features + per-strip one-column
        matmuls (node-major), inverse norms via Ln/Exp(-0.5);
      - norm rows bounced through DRAM to node-row layout, broadcast across
        partitions on GpSimd, features normalized on DVE;
      - per-strip 128x128 gram blocks (normalized + unnormalized) with masked
        pair reductions via scalar_tensor_tensor / activation accumulators;
      - per-fragment sums via one-hot matmuls; per-core partials -> [1,3].
  * Host combines 8 partials (sum + the two global Frobenius sqrts).

All pairwise math uses the identity  sum_pm (S-t)^2 = A - 2tB + t^2*npairs
with A = sum pm*S^2, B = sum pm*S, so no (S-t) intermediates are built.
"""
import numpy as np
import ml_dtypes
from contextlib import ExitStack

BF = ml_dtypes.bfloat16

# ---------------- problem constants (hardcoded per contract) ----------------
N, D, NF = 8192, 256, 256
R = 0.3
BRICS = 0.4
AW = 0.6
VW = 0.2
CF = 0.1
NCORES = 8
NFC = NF // NCORES          # 32 fragments per core
P = 128                     # strip height
NSTRIP = 9                  # strips per core
M = NSTRIP * P              # 1152 padded slots per core
CH = 384                    # encoder column chunk
NCH = M // CH
LN2 = np.float32(np.log(2.0))

# Q column indices (per-strip accumulator tile)
(QB_SH, QA_SH, QA_UN, QC_UN, QB_DIR, QA_DIR, QSOFF, QVOFF, QJK) = range(9)
NQ = 9
# cvec columns: 0..8 coeffs for Q cols, 9..13 weights for segn (nsq per set),
# 14 = additive const
NCV = 15

# weight-pack column offsets in the [128, 6144] packed weight tensor
_WOFF = {}
_off = 0
for _nm, _kt, _cols in [("w1c", 4, D), ("w2", 2, D), ("wd1", 2, 2 * D),
                        ("wd2", 4, 2 * D), ("wv1c", 4, D), ("wv2", 2, D)]:
    _WOFF[_nm] = (_off, _kt, _cols)
    _off += _kt * _cols
WCOLS = _off  # 6144


# ============================ host-side prep ================================

def _assign_fragments(fid):
    counts = np.bincount(fid, minlength=NF)
    order = np.argsort(-counts, kind="stable")
    core_frags = [[] for _ in range(NCORES)]
    for i, f in enumerate(order):
        r = i // NCORES
        c = i % NCORES if r % 2 == 0 else NCORES - 1 - (i % NCORES)
        core_frags[c].append(int(f))
    layout = []
    for c in range(NCORES):
        frags = sorted(core_frags[c], key=lambda f: -counts[f])
        strips = [[] for _ in range(NSTRIP)]
        fill = np.zeros(NSTRIP, dtype=int)
        for f in frags:
            for s in range(NSTRIP):
                if fill[s] + counts[f] <= P:
                    strips[s].append(f)
                    fill[s] += counts[f]
                    break
            else:
                raise AssertionError(f"core {c}: fragment {f} does not fit")
        layout.append(strips)
    return layout, counts


def _build_core_meta(fid, atom_types, layout, counts):
    nodes_of = {f: np.nonzero(fid == f)[0] for f in range(NF)}
    metas = []
    for c in range(NCORES):
        slot_node = -np.ones(M, dtype=np.int64)
        slot_frag = -np.ones(M, dtype=np.int64)
        frag_global = []
        fl = 0
        for b in range(NSTRIP):
            pos = b * P
            for f in layout[c][b]:
                nn = nodes_of[f]
                slot_node[pos:pos + len(nn)] = nn
                slot_frag[pos:pos + len(nn)] = fl
                frag_global.append(f)
                pos += len(nn)
                fl += 1
        assert fl == NFC
        frag_global = np.array(frag_global, dtype=np.int64)
        real = slot_node >= 0

        pm3 = np.zeros((NSTRIP, P, 3 * P), dtype=np.float32)
        tm = np.zeros((NSTRIP, P, P), dtype=np.float32)
        amat = np.zeros((NSTRIP, P, NFC), dtype=np.float32)
        t2pm_frag = np.zeros(NFC, dtype=np.float32)
        for b in range(NSTRIP):
            sf = slot_frag[b * P:(b + 1) * P]
            sn = slot_node[b * P:(b + 1) * P]
            rr = sf >= 0
            same = (sf[:, None] == sf[None, :]) & rr[:, None] & rr[None, :]
            upper = np.triu(np.ones((P, P), dtype=bool), k=1)
            pmb = (same & upper).astype(np.float32)
            pm3[b] = np.tile(pmb, (1, 3))
            at = np.where(rr, atom_types[np.where(rr, sn, 0)], -1)
            tgt = np.where(at[:, None] == at[None, :], 0.3, 0.1).astype(np.float32)
            tm[b] = tgt * pmb
            for p in range(P):
                if rr[p]:
                    amat[b, p, sf[p]] = 1.0
            for fl_ in np.unique(sf[rr]):
                sel = sf == fl_
                t2pm_frag[fl_] += float(((tgt * pmb) ** 2)[np.ix_(sel, sel)].sum())

        cnt = counts[frag_global].astype(np.float32)
        valid = (cnt >= 2.0).astype(np.float32)
        pairs = cnt * (cnt - 1.0) * 0.5
        safe_c = np.maximum(cnt, 1.0)
        safe_p = np.maximum(pairs, 1.0)

        t_sh = np.float32(BRICS)
        t_dir = np.float32(BRICS - 0.2)
        cvec = np.zeros((NFC, NCV), dtype=np.float32)
        cvec[:, QB_SH] = -2.0 * t_sh * AW / safe_p
        cvec[:, QA_SH] = AW / safe_p
        cvec[:, QA_UN] = (1.0 - AW) / safe_p
        cvec[:, QC_UN] = -2.0 * (1.0 - AW) / safe_p
        cvec[:, QB_DIR] = -2.0 * t_dir * VW / (3.0 * safe_p)
        cvec[:, QA_DIR] = VW / (3.0 * safe_p)
        cvec[:, QSOFF] = -2.0 * AW / (safe_c * safe_c)
        cvec[:, QVOFF] = -2.0 * VW / (safe_c * safe_c)
        cvec[:, QJK] = 0.0
        cvec[:, 9] = AW * (1.0 / safe_c - 1.0 / (safe_c * safe_c))    # nsq_sh
        cvec[:, 10] = 0.0                                             # nsq_un
        wv_ = VW * (1.0 / safe_c - 1.0 / (safe_c * safe_c))
        cvec[:, 11] = wv_
        cvec[:, 12] = wv_
        cvec[:, 13] = wv_
        cvec[:, 14] = (AW * t_sh * t_sh * pairs / safe_p
                       + (1.0 - AW) * t2pm_frag / safe_p
                       + VW * t_dir * t_dir * pairs / safe_p)
        cvec *= valid[:, None]

        metas.append(dict(slot_node=slot_node, real=real, pm3=pm3, tm=tm,
                          amat=amat, cvec=cvec))
    return metas


def _pack_rows(x, kt):
    # [kt*128, cols] -> [128, kt*cols] so each partition's data is contiguous
    cols = x.shape[1]
    return np.ascontiguousarray(
        x.reshape(kt, P, cols).transpose(1, 0, 2).reshape(P, kt * cols))


def _shard_inputs(inputs):
    fid = np.asarray(inputs["fragment_ids"]).astype(np.int64)
    at = np.asarray(inputs["atom_types"]).astype(np.int64)
    layout, counts = _assign_fragments(fid)
    metas = _build_core_meta(fid, at, layout, counts)

    W1 = np.asarray(inputs["W1"], np.float32)
    W2 = np.asarray(inputs["W2"], np.float32)
    Wd1 = np.asarray(inputs["Wd1"], np.float32)
    Wd2 = np.asarray(inputs["Wd2"], np.float32)
    Wv1 = np.asarray(inputs["Wv1"], np.float32)
    Wv2 = np.asarray(inputs["Wv2"], np.float32)
    w1c = np.concatenate([R * W1, (1.0 - R) * W1], axis=0)
    wv1c = np.concatenate([R * Wv1, (1.0 - R) * Wv1], axis=0)
    wall = np.concatenate([
        _pack_rows(w1c, 4), _pack_rows(W2, 2), _pack_rows(Wd1, 2),
        _pack_rows(Wd2, 4), _pack_rows(wv1c, 4), _pack_rows(Wv2, 2)],
        axis=1).astype(BF)
    assert wall.shape == (P, WCOLS)

    b2p = (np.asarray(inputs["b2"], np.float32) - LN2 * W2.sum(axis=0))
    bd2p = (np.asarray(inputs["bd2"], np.float32) - LN2 * Wd2.sum(axis=0))
    bv2p = (-LN2 * Wv2.sum(axis=0))
    ball = np.concatenate([
        np.asarray(inputs["b1"], np.float32).reshape(2, P).T,
        b2p.reshape(2, P).T,
        np.asarray(inputs["bd1"], np.float32).reshape(4, P).T,
        bd2p.reshape(4, P).T,
        bv2p.reshape(2, P).T], axis=1).astype(np.float32)
    assert ball.shape == (P, 14)

    ss = np.asarray(inputs["scalar_short"], np.float32)
    sl = np.asarray(inputs["scalar_long"], np.float32)
    vs = np.asarray(inputs["vector_short"], np.float32)
    vl = np.asarray(inputs["vector_long"], np.float32)

    in_maps = []
    for c in range(NCORES):
        m = metas[c]
        idx = np.where(m["real"], m["slot_node"], 0)

        def take2(x):
            g = (x[idx] * m["real"][:, None]).astype(np.float32)
            return _pack_rows(np.ascontiguousarray(g.T), 2).astype(BF)

        def take3(x):
            g = (x[idx] * m["real"][:, None, None]).astype(np.float32)
            t = np.ascontiguousarray(g.transpose(1, 2, 0).reshape(3 * D, M))
            return _pack_rows(t, 6).astype(BF)

        in_maps.append({
            "x_s": take2(ss), "x_l": take2(sl),
            "v_s": take3(vs), "v_l": take3(vl),
            "wall": wall, "ball": ball,
            "pm3": np.ascontiguousarray(
                m["pm3"].transpose(1, 0, 2).reshape(P, NSTRIP * 3 * P)).astype(BF),
            "tm": np.ascontiguousarray(
                m["tm"].transpose(1, 0, 2).reshape(P, NSTRIP * P)).astype(BF),
            "amat": np.ascontiguousarray(
                m["amat"].transpose(1, 0, 2).reshape(P, NSTRIP * NFC)),
            "cvec": m["cvec"],
            "ident": np.ascontiguousarray(np.eye(P, dtype=np.float32)).astype(BF),
        })
    n_valid = float((counts >= 2).sum())
    return in_maps, n_valid


def _combine(fins, n_valid):
    loss = float(sum(float(f[0]) for f in fins))
    ssq_sh = float(sum(float(f[1]) for f in fins))
    ssq_un = float(sum(float(f[2]) for f in fins))
    l2 = 0.01 * (np.sqrt(ssq_sh) + np.sqrt(ssq_un))
    if n_valid > 0:
        return np.float32(CF * (loss + n_valid * l2) / max(n_valid, 1.0))
    return np.float32(0.0)


# ============================ device program ================================

_NC_CACHE = {}


def build_nc(debug_out=False):
    key = ("nc", debug_out)
    if key in _NC_CACHE:
        return _NC_CACHE[key]
    import concourse.bass as bass
    import concourse.bacc as bacc
    import concourse.mybir as mybir
    import concourse.tile as tile

    F32 = mybir.dt.float32
    BF16 = mybir.dt.bfloat16
    AF = mybir.ActivationFunctionType
    ALU = mybir.AluOpType

    nc = bacc.Bacc("TRN2", target_bir_lowering=False, debug=False)

    d = {}
    for name, shape, dt_ in [
        ("x_s", [P, 2 * M], BF16), ("x_l", [P, 2 * M], BF16),
        ("v_s", [P, 6 * M], BF16), ("v_l", [P, 6 * M], BF16),
        ("wall", [P, WCOLS], BF16), ("ball", [P, 14], F32),
        ("pm3", [P, NSTRIP * 3 * P], BF16), ("tm", [P, NSTRIP * P], BF16),
        ("amat", [P, NSTRIP * NFC], F32), ("cvec", [NFC, NCV], F32),
        ("ident", [P, P], BF16),
    ]:
        d[name] = nc.dram_tensor(name, shape, dt_, kind="ExternalInput").ap()
    d_out = nc.dram_tensor("out", [1, 3], F32, kind="ExternalOutput").ap()
    d_inv = nc.dram_tensor("inv_scratch", [NCH * 15, P], BF16).ap()
    d_dbg = {}
    if debug_out:
        for nm, shape, dt_ in [("dbg_nsq", [P, 45], F32),
                               ("dbg_seg", [NFC, NQ], F32),
                               ("dbg_segn", [NFC, 5], F32),
                               ("dbg_irow", [1, 5 * M], F32)]:
            d_dbg[nm] = nc.dram_tensor(nm, shape, dt_, kind="ExternalOutput").ap()

    with tile.TileContext(nc) as tc, ExitStack() as ctx:
        wpool = ctx.enter_context(tc.tile_pool(name="w", bufs=1))
        feat = ctx.enter_context(tc.tile_pool(name="feat", bufs=1))
        xin = ctx.enter_context(tc.tile_pool(name="xin", bufs=2))
        small = ctx.enter_context(tc.tile_pool(name="small", bufs=1))
        rowp = ctx.enter_context(tc.tile_pool(name="rowp", bufs=2))
        junkp = ctx.enter_context(tc.tile_pool(name="junk", bufs=3))
        psp = ctx.enter_context(tc.tile_pool(name="ps", bufs=4, space="PSUM"))
        pnsq = ctx.enter_context(tc.tile_pool(name="pnsq", bufs=1, space="PSUM"))
        pseg = ctx.enter_context(tc.tile_pool(name="pseg", bufs=1, space="PSUM"))

        # ---- constants / weights (one DMA each); encoder inputs issue first
        # on the sync queue, G-phase masks go on the scalar queue ----
        def ld(name, shape, dt_, pool=wpool, eng=None):
            t = pool.tile(shape, dt_, tag=name, name=name)
            (eng or nc.sync).dma_start(out=t, in_=d[name])
            return t

        wall_sb = wpool.tile([P, WCOLS], BF16, tag="wall", name="wall")
        nc.sync.dma_start(out=wall_sb[:, 0:1536], in_=d["wall"][:, 0:1536])
        ball_sb = ld("ball", [P, 14], F32)
        xs_sb = ld("x_s", [P, 2 * M], BF16, pool=feat)
        xl_sb = ld("x_l", [P, 2 * M], BF16, pool=feat)
        nc.sync.dma_start(out=wall_sb[:, 1536:WCOLS], in_=d["wall"][:, 1536:WCOLS])
        vs_sb = ld("v_s", [P, 6 * M], BF16, pool=feat)
        vl_sb = ld("v_l", [P, 6 * M], BF16, pool=feat)
        pm3_sb = ld("pm3", [P, NSTRIP * 3 * P], BF16, eng=nc.scalar)
        tm_sb = ld("tm", [P, NSTRIP * P], BF16, eng=nc.scalar)
        amat_sb = ld("amat", [P, NSTRIP * NFC], F32, eng=nc.scalar)
        cvec_sb = ld("cvec", [NFC, NCV], F32, eng=nc.scalar)
        ident_sb = ld("ident", [P, P], BF16, eng=nc.scalar)

        def W(name, k):
            off, kt, cols = _WOFF[name]
            assert k < kt
            return wall_sb[:, off + k * cols: off + (k + 1) * cols]

        def bias(name, m_):
            offs = {"b1": 0, "b2p": 2, "bd1": 4, "bd2p": 8, "bv2p": 12}
            return ball_sb[:, offs[name] + m_: offs[name] + m_ + 1]

        ones_col = wpool.tile([P, 1], BF16, tag="ones_col", name="ones_col")
        nc.vector.memset(ones_col, 1.0)
        ones32 = wpool.tile([NFC, 1], F32, tag="ones32", name="ones32")
        nc.vector.memset(ones32, 1.0)
        eps_sb = wpool.tile([P, 1], F32, tag="eps", name="eps")
        nc.vector.memset(eps_sb, 1e-20)

        # ---- persistent per-chunk feature tiles ----
        # sets order: 0=sh 1=un 2=v0 3=v1 4=v2 ; each set has k-tiles 0,1
        u_t = [[[feat.tile([P, CH], BF16, tag=f"u{s}_{k}_{c}", name=f"u{s}_{k}_{c}")
                 for c in range(NCH)] for k in range(2)] for s in range(5)]
        n_t = [[[feat.tile([P, CH], BF16, tag=f"n{s}_{k}_{c}", name=f"n{s}_{k}_{c}")
                 for c in range(NCH)] for k in range(2)] for s in range(5)]

        nsq_ps = pnsq.tile([P, 45], F32, tag="nsq", name="nsq_ps")

        def mm_chunk(pt, w_name, rhs_list, m_):
            kt = len(rhs_list)
            for k in range(kt):
                nc.tensor.matmul(pt[:, 0:CH], W(w_name, k)[:, m_ * P:(m_ + 1) * P],
                                 rhs_list[k], start=(k == 0), stop=(k == kt - 1))

        def xsl(t, k, c):
            return t[:, k * M + c * CH: k * M + (c + 1) * CH]

        # ---- encoder phase A: h1 = softplus(x@W1c + b1), split Exp/Ln ----
        e_h1, t_h1, s_all = [], [], []
        for c in range(NCH):
            ew = xin.tile([P, 2 * CH], BF16, tag="eh1", name=f"eh1_{c}", bufs=2)
            for m_ in range(2):
                pt = psp.tile([P, 512], F32, tag="ps", name="ps")
                mm_chunk(pt, "w1c", [xsl(xs_sb, 0, c), xsl(xs_sb, 1, c),
                                     xsl(xl_sb, 0, c), xsl(xl_sb, 1, c)], m_)
                nc.scalar.activation(ew[:, m_ * CH:(m_ + 1) * CH], pt[:, 0:CH],
                                     AF.Exp, bias=bias("b1", m_))
            e_h1.append(ew)
        for c in range(NCH):
            tw = xin.tile([P, 2 * CH], BF16, tag="th1", name=f"th1_{c}", bufs=2)
            nc.scalar.activation(tw, e_h1[c], AF.Ln, bias=1.0)
            t_h1.append(tw)
        for c in range(NCH):
            s_ = []
            for m_ in range(2):
                pt = psp.tile([P, 512], F32, tag="ps", name="ps")
                mm_chunk(pt, "w2", [t_h1[c][:, 0:CH], t_h1[c][:, CH:2 * CH]], m_)
                t = xin.tile([P, CH], BF16, tag=f"s{m_}", name=f"s{m_}_{c}", bufs=2)
                nc.vector.tensor_scalar_add(t, pt[:, 0:CH], bias("b2p", m_))
                s_.append(t)
            s_all.append(s_)

        # ---- encoder phase B: hd + v1 exps, wide Ln, then dec/v2 evacs ----
        e_B, t_B = [], []
        for c in range(NCH):
            ew = xin.tile([P, 10 * CH], BF16, tag="eB", name=f"eB_{c}", bufs=2)
            for m_ in range(4):
                pt = psp.tile([P, 512], F32, tag="ps", name="ps")
                mm_chunk(pt, "wd1", s_all[c], m_)
                nc.scalar.activation(ew[:, m_ * CH:(m_ + 1) * CH], pt[:, 0:CH],
                                     AF.Exp, bias=bias("bd1", m_))
            for dd in range(3):
                rhs = [xsl(vs_sb, 2 * dd, c), xsl(vs_sb, 2 * dd + 1, c),
                       xsl(vl_sb, 2 * dd, c), xsl(vl_sb, 2 * dd + 1, c)]
                for m_ in range(2):
                    pt = psp.tile([P, 512], F32, tag="ps", name="ps")
                    mm_chunk(pt, "wv1c", rhs, m_)
                    j = 4 + 2 * dd + m_
                    nc.scalar.activation(ew[:, j * CH:(j + 1) * CH], pt[:, 0:CH],
                                         AF.Exp)
            e_B.append(ew)
        for c in range(NCH):
            tw = xin.tile([P, 10 * CH], BF16, tag="tB", name=f"tB_{c}", bufs=2)
            nc.scalar.activation(tw, e_B[c], AF.Ln, bias=1.0)
            t_B.append(tw)

        xsq_t = [[None] * 2 for _ in range(5)]

        def sq_and_nsq(s, k, c, src):
            # xsq = src^2; the one-column matmuls that reduce it to nsq are
            # emitted after the chunk's evacs so each column's two-matmul
            # accumulation group is contiguous
            t = feat.tile([P, CH], BF16, tag=f"xsq{s}_{k}", name=f"xsq{s}_{k}")
            if s < 2:
                nc.gpsimd.tensor_mul(t, src, src)
            else:
                nc.vector.tensor_mul(t, src, src)
            xsq_t[s][k] = t

        def nsq_mms(c):
            # column layout is (chunk, set, local-strip)-major so each chunk's
            # 15 columns are contiguous for the transpose
            for s in range(5):
                for ls in range(3):
                    col = c * 15 + s * 3 + ls
                    for k in range(2):
                        nc.tensor.matmul(
                            nsq_ps[:, col:col + 1],
                            xsq_t[s][k][:, ls * P:(ls + 1) * P], ones_col,
                            start=(k == 0), stop=(k == 1))

        def msum(out_t, in0, in1, acc):
            nc.vector.scalar_tensor_tensor(
                out=out_t, in0=in0, scalar=1.0, in1=in1,
                op0=ALU.bypass, op1=ALU.mult, accum_out=acc)

        Q_b = [small.tile([P, NQ], F32, tag=f"Q{b}", name=f"Q{b}")
               for b in range(NSTRIP)]
        d_inv_rd = d_inv.rearrange("(o c r) p -> c o (r p)", o=1, c=NCH, r=15)

        def strip_work(b):
            cb, ls = b // 3, (b % 3)
            lsl = slice(ls * P, (ls + 1) * P)
            pmb = pm3_sb[:, b * 3 * P: b * 3 * P + P]
            pm3b = pm3_sb[:, b * 3 * P: (b + 1) * 3 * P]
            tmb = tm_sb[:, b * P:(b + 1) * P]
            Q = Q_b[b]

            def gpair(pt, col0, tiles):
                for k in range(2):
                    nc.tensor.matmul(pt[:, col0:col0 + P], tiles[k][cb][:, lsl],
                                     tiles[k][cb][:, lsl],
                                     start=(k == 0), stop=(k == 1))

            # unnormalized grams (sh + 3 v dims) packed in one bank
            gu = psp.tile([P, 512], F32, tag="ps", name="gu")
            gpair(gu, 0, u_t[0])
            for dd in range(3):
                gpair(gu, P + dd * P, u_t[2 + dd])
            j5 = junkp.tile([P, P], BF16, tag="jk1", name="j5")
            msum(j5, gu[:, 0:P], pmb, Q[:, QSOFF:QSOFF + 1])
            j6 = junkp.tile([P, 3 * P], BF16, tag="jk3", name="j6")
            msum(j6, gu[:, P:4 * P], pm3b, Q[:, QVOFF:QVOFF + 1])

            # normalized grams: [sh | un] bank and [v0 | v1 | v2] bank
            gn = psp.tile([P, 512], F32, tag="ps", name="gn")
            gpair(gn, 0, n_t[0])
            gpair(gn, P, n_t[1])
            gv = psp.tile([P, 512], F32, tag="ps", name="gv")
            for dd in range(3):
                gpair(gv, dd * P, n_t[2 + dd])

            spm = junkp.tile([P, P], BF16, tag="spm", name="spm", bufs=2)
            msum(spm, gn[:, 0:P], pmb, Q[:, QB_SH:QB_SH + 1])
            nc.scalar.activation(junkp.tile([P, P], BF16, tag="jk1", name="j1"),
                                 spm, AF.Square, accum_out=Q[:, QA_SH:QA_SH + 1])
            upm = junkp.tile([P, P], BF16, tag="upm", name="upm", bufs=2)
            msum(upm, gn[:, P:2 * P], pmb, Q[:, QJK:QJK + 1])
            nc.scalar.activation(junkp.tile([P, P], BF16, tag="jk1", name="j2"),
                                 upm, AF.Square, accum_out=Q[:, QA_UN:QA_UN + 1])
            j3 = junkp.tile([P, P], BF16, tag="jk1", name="j3")
            msum(j3, gn[:, P:2 * P], tmb, Q[:, QC_UN:QC_UN + 1])

            dpm = junkp.tile([P, 3 * P], BF16, tag="dpm", name="dpm", bufs=2)
            msum(dpm, gv[:, 0:3 * P], pm3b, Q[:, QB_DIR:QB_DIR + 1])
            nc.scalar.activation(junkp.tile([P, 3 * P], BF16, tag="jk3", name="j4"),
                                 dpm, AF.Square, accum_out=Q[:, QA_DIR:QA_DIR + 1])

        # dec/v2 evacs (DVE) + squared features + per-strip nsq columns
        for c in range(NCH):
            hd = [t_B[c][:, m_ * CH:(m_ + 1) * CH] for m_ in range(4)]
            for m_ in range(4):
                pt = psp.tile([P, 512], F32, tag="ps", name="ps")
                mm_chunk(pt, "wd2", hd, m_)
                s, k = (0, m_) if m_ < 2 else (1, m_ - 2)
                nc.vector.tensor_scalar_add(u_t[s][k][c], pt[:, 0:CH],
                                            bias("bd2p", m_))
                sq_and_nsq(s, k, c, u_t[s][k][c])
            for dd in range(3):
                v1 = [t_B[c][:, (4 + 2 * dd + k) * CH:(5 + 2 * dd + k) * CH]
                      for k in range(2)]
                for m_ in range(2):
                    pt = psp.tile([P, 512], F32, tag="ps", name="ps")
                    mm_chunk(pt, "wv2", v1, m_)
                    nc.vector.tensor_scalar_add(u_t[2 + dd][m_][c], pt[:, 0:CH],
                                                bias("bv2p", m_))
                    sq_and_nsq(2 + dd, m_, c, u_t[2 + dd][m_][c])
            nsq_mms(c)

        # ---- inverse norms: inv = (nsq+eps)^(-1/2); transpose on the tensor
        # engine to node-row layout so the DRAM bounce is contiguous ----
        nsq_sb = small.tile([P, 45], F32, tag="nsq_sb", name="nsq_sb")
        nc.vector.tensor_copy(nsq_sb, nsq_ps)
        lnn = small.tile([P, 45], F32, tag="lnn", name="lnn")
        nc.scalar.activation(lnn, nsq_ps, AF.Ln, bias=eps_sb[:, 0:1])
        invN = small.tile([P, 45], BF16, tag="invN", name="invN")
        nc.scalar.activation(invN, lnn, AF.Exp, scale=-0.5)
        for c in range(NCH):
            invc = invN[:, c * 15:(c + 1) * 15]
            invT_ps = pseg.tile([15, P], BF16, tag="invT", name="invT")
            nc.tensor.transpose(invT_ps, invc, ident_sb)
            invT_c = junkp.tile([15, P], BF16, tag="invT_sb", name=f"invT{c}",
                                bufs=2)
            nc.vector.tensor_copy(invT_c, invT_ps)
            nc.sync.dma_start(out=d_inv[c * 15:(c + 1) * 15, :], in_=invT_c)

        # normalize + per-strip grams/reductions, chunk by chunk
        for c in range(NCH):
            irow_c = rowp.tile([1, 15 * P], BF16, tag="irow", name=f"irow{c}",
                               bufs=2)
            nc.sync.dma_start(out=irow_c, in_=d_inv_rd[c])
            for s in range(5):
                B_sc = rowp.tile([P, CH], BF16, tag="Bb", name=f"B{s}_{c}", bufs=3)
                nc.gpsimd.partition_broadcast(
                    B_sc, irow_c[0:1, s * CH:(s + 1) * CH])
                for k in range(2):
                    nc.vector.tensor_mul(n_t[s][k][c], u_t[s][k][c], B_sc)
        for b in range(NSTRIP):
            strip_work(b)

        # ---- segment reduction + final combine ----
        segq_ps = pseg.tile([NFC, NQ + 5], F32, tag="seg", name="seg")
        seg_ps = segq_ps[:, 0:NQ]
        segn_ps = segq_ps[:, NQ:NQ + 5]
        nsq_v = nsq_sb.rearrange("p (c s b) -> p c s b", c=NCH, s=5)
        for b in range(NSTRIP):
            nc.tensor.matmul(seg_ps, amat_sb[:, b * NFC:(b + 1) * NFC], Q_b[b],
                             start=(b == 0), stop=(b == NSTRIP - 1))
        for b in range(NSTRIP):
            nc.tensor.matmul(segn_ps, amat_sb[:, b * NFC:(b + 1) * NFC],
                             nsq_v[:, b // 3, :, b % 3],
                             start=(b == 0), stop=(b == NSTRIP - 1))
        segq = small.tile([NFC, NQ + 5], F32, tag="segs", name="segs")
        nc.vector.tensor_copy(segq, segq_ps)
        segs = segq[:, 0:NQ]
        segn = segq[:, NQ:NQ + 5]

        acc = small.tile([NFC, 3], F32, tag="acc", name="acc")
        junkq = small.tile([NFC, NQ + 5], F32, tag="junkq", name="junkq")
        # acc0 = sum_q cvec[q]*segq[q] over Q cols + segn cols, then + const
        nc.vector.scalar_tensor_tensor(
            out=junkq, in0=segq, scalar=1.0, in1=cvec_sb[:, 0:NQ + 5],
            op0=ALU.bypass, op1=ALU.mult, accum_out=acc[:, 0:1])
        nc.vector.tensor_add(acc[:, 0:1], acc[:, 0:1], cvec_sb[:, 14:15])
        nc.vector.tensor_copy(acc[:, 1:2], segn[:, 0:1])
        nc.vector.tensor_copy(acc[:, 2:3], segn[:, 1:2])

        if debug_out:
            nc.sync.dma_start(out=d_dbg["dbg_nsq"], in_=nsq_sb)
            nc.sync.dma_start(out=d_dbg["dbg_seg"], in_=segs)
            nc.sync.dma_start(out=d_dbg["dbg_segn"], in_=segn)
            irow_f = rowp.tile([1, 5 * M], F32, tag="irow_f", name="irow_f", bufs=1)
            nc.vector.tensor_copy(irow_f, irow)
            nc.sync.dma_start(out=d_dbg["dbg_irow"], in_=irow_f)

        fin_ps = pseg.tile([1, 3], F32, tag="fin", name="fin")
        nc.tensor.matmul(fin_ps, ones32, acc, start=True, stop=True)
        fin_sb = small.tile([1, 3], F32, tag="fin_sb", name="fin_sb")
        nc.vector.tensor_copy(fin_sb, fin_ps)
        nc.sync.dma_start(out=d_out, in_=fin_sb)

    nc.compile()
    _NC_CACHE[key] = nc
    return nc


# ============================== entry point =================================

def kernel(**inputs) -> np.ndarray:
    from concourse.bass_utils import run_bass_kernel_spmd

    in_maps, n_valid = _shard_inputs(inputs)
    nc = build_nc()
    res = run_bass_kernel_spmd(nc, in_maps, core_ids=list(range(NCORES)))
    fins = [r["out"].reshape(3) for r in res.results]
    return _combine(fins, n_valid)


# revision 37
# speedup vs baseline: 1.3795x; 1.0070x over previous
"""Trainium2 Bass kernel for BalancedConformationalConsistencyLoss.

Strategy (segment/fragment parallelism, 8 cores):
  * Host sorts nodes by fragment; 32 fragments per core (snake-deal by size),
    bin-packed into 9 strips of 128 slots so no fragment straddles a strip.
  * Device (per core, SPMD), all-bf16 matmul path:
      - encoder MLPs feature-major, softplus split into Exp/Ln phases batched
        by activation function (5 table loads total);
      - per-node squared norms via squared-features + per-strip one-column
        matmuls (node-major), inverse norms via Ln/Exp(-0.5);
      - norm rows bounced through DRAM to node-row layout, broadcast across
        partitions on GpSimd, features normalized on DVE;
      - per-strip 128x128 gram blocks (normalized + unnormalized) with masked
        pair reductions via scalar_tensor_tensor / activation accumulators;
      - per-fragment sums via one-hot matmuls; per-core partials -> [1,3].
  * Host combines 8 partials (sum + the two global Frobenius sqrts).

All pairwise math uses the identity  sum_pm (S-t)^2 = A - 2tB + t^2*npairs
with A = sum pm*S^2, B = sum pm*S, so no (S-t) intermediates are built.
"""
import numpy as np
import ml_dtypes
from contextlib import ExitStack

BF = ml_dtypes.bfloat16

# ---------------- problem constants (hardcoded per contract) ----------------
N, D, NF = 8192, 256, 256
R = 0.3
BRICS = 0.4
AW = 0.6
VW = 0.2
CF = 0.1
NCORES = 8
NFC = NF // NCORES          # 32 fragments per core
P = 128                     # strip height
NSTRIP = 9                  # strips per core
M = NSTRIP * P              # 1152 padded slots per core
CH = 384                    # encoder column chunk
NCH = M // CH
LN2 = np.float32(np.log(2.0))

# Q column indices (per-strip accumulator tile)
(QB_SH, QA_SH, QA_UN, QC_UN, QB_DIR, QA_DIR, QSOFF, QVOFF, QJK) = range(9)
NQ = 9
# cvec columns: 0..8 coeffs for Q cols, 9..13 weights for segn (nsq per set),
# 14 = additive const
NCV = 15

# weight-pack column offsets in the [128, 6144] packed weight tensor
_WOFF = {}
_off = 0
for _nm, _kt, _cols in [("w1c", 4, D), ("w2", 2, D), ("wd1", 2, 2 * D),
                        ("wd2", 4, 2 * D), ("wv1c", 4, D), ("wv2", 2, D)]:
    _WOFF[_nm] = (_off, _kt, _cols)
    _off += _kt * _cols
WCOLS = _off  # 6144


# ============================ host-side prep ================================

def _assign_fragments(fid):
    counts = np.bincount(fid, minlength=NF)
    order = np.argsort(-counts, kind="stable")
    core_frags = [[] for _ in range(NCORES)]
    for i, f in enumerate(order):
        r = i // NCORES
        c = i % NCORES if r % 2 == 0 else NCORES - 1 - (i % NCORES)
        core_frags[c].append(int(f))
    layout = []
    for c in range(NCORES):
        frags = sorted(core_frags[c], key=lambda f: -counts[f])
        strips = [[] for _ in range(NSTRIP)]
        fill = np.zeros(NSTRIP, dtype=int)
        for f in frags:
            for s in range(NSTRIP):
                if fill[s] + counts[f] <= P:
                    strips[s].append(f)
                    fill[s] += counts[f]
                    break
            else:
                raise AssertionError(f"core {c}: fragment {f} does not fit")
        layout.append(strips)
    return layout, counts


def _build_core_meta(fid, atom_types, layout, counts):
    nodes_of = {f: np.nonzero(fid == f)[0] for f in range(NF)}
    metas = []
    for c in range(NCORES):
        slot_node = -np.ones(M, dtype=np.int64)
        slot_frag = -np.ones(M, dtype=np.int64)
        frag_global = []
        fl = 0
        for b in range(NSTRIP):
            pos = b * P
            for f in layout[c][b]:
                nn = nodes_of[f]
                slot_node[pos:pos + len(nn)] = nn
                slot_frag[pos:pos + len(nn)] = fl
                frag_global.append(f)
                pos += len(nn)
                fl += 1
        assert fl == NFC
        frag_global = np.array(frag_global, dtype=np.int64)
        real = slot_node >= 0

        pm3 = np.zeros((NSTRIP, P, 3 * P), dtype=np.float32)
        tm = np.zeros((NSTRIP, P, P), dtype=np.float32)
        amat = np.zeros((NSTRIP, P, NFC), dtype=np.float32)
        t2pm_frag = np.zeros(NFC, dtype=np.float32)
        for b in range(NSTRIP):
            sf = slot_frag[b * P:(b + 1) * P]
            sn = slot_node[b * P:(b + 1) * P]
            rr = sf >= 0
            same = (sf[:, None] == sf[None, :]) & rr[:, None] & rr[None, :]
            upper = np.triu(np.ones((P, P), dtype=bool), k=1)
            pmb = (same & upper).astype(np.float32)
            pm3[b] = np.tile(pmb, (1, 3))
            at = np.where(rr, atom_types[np.where(rr, sn, 0)], -1)
            tgt = np.where(at[:, None] == at[None, :], 0.3, 0.1).astype(np.float32)
            tm[b] = tgt * pmb
            for p in range(P):
                if rr[p]:
                    amat[b, p, sf[p]] = 1.0
            for fl_ in np.unique(sf[rr]):
                sel = sf == fl_
                t2pm_frag[fl_] += float(((tgt * pmb) ** 2)[np.ix_(sel, sel)].sum())

        cnt = counts[frag_global].astype(np.float32)
        valid = (cnt >= 2.0).astype(np.float32)
        pairs = cnt * (cnt - 1.0) * 0.5
        safe_c = np.maximum(cnt, 1.0)
        safe_p = np.maximum(pairs, 1.0)

        t_sh = np.float32(BRICS)
        t_dir = np.float32(BRICS - 0.2)
        cvec = np.zeros((NFC, NCV), dtype=np.float32)
        cvec[:, QB_SH] = -2.0 * t_sh * AW / safe_p
        cvec[:, QA_SH] = AW / safe_p
        cvec[:, QA_UN] = (1.0 - AW) / safe_p
        cvec[:, QC_UN] = -2.0 * (1.0 - AW) / safe_p
        cvec[:, QB_DIR] = -2.0 * t_dir * VW / (3.0 * safe_p)
        cvec[:, QA_DIR] = VW / (3.0 * safe_p)
        cvec[:, QSOFF] = -2.0 * AW / (safe_c * safe_c)
        cvec[:, QVOFF] = -2.0 * VW / (safe_c * safe_c)
        cvec[:, QJK] = 0.0
        cvec[:, 9] = AW * (1.0 / safe_c - 1.0 / (safe_c * safe_c))    # nsq_sh
        cvec[:, 10] = 0.0                                             # nsq_un
        wv_ = VW * (1.0 / safe_c - 1.0 / (safe_c * safe_c))
        cvec[:, 11] = wv_
        cvec[:, 12] = wv_
        cvec[:, 13] = wv_
        cvec[:, 14] = (AW * t_sh * t_sh * pairs / safe_p
                       + (1.0 - AW) * t2pm_frag / safe_p
                       + VW * t_dir * t_dir * pairs / safe_p)
        cvec *= valid[:, None]

        metas.append(dict(slot_node=slot_node, real=real, pm3=pm3, tm=tm,
                          amat=amat, cvec=cvec))
    return metas


def _pack_rows(x, kt):
    # [kt*128, cols] -> [128, kt*cols] so each partition's data is contiguous
    cols = x.shape[1]
    return np.ascontiguousarray(
        x.reshape(kt, P, cols).transpose(1, 0, 2).reshape(P, kt * cols))


def _shard_inputs(inputs):
    fid = np.asarray(inputs["fragment_ids"]).astype(np.int64)
    at = np.asarray(inputs["atom_types"]).astype(np.int64)
    layout, counts = _assign_fragments(fid)
    metas = _build_core_meta(fid, at, layout, counts)

    W1 = np.asarray(inputs["W1"], np.float32)
    W2 = np.asarray(inputs["W2"], np.float32)
    Wd1 = np.asarray(inputs["Wd1"], np.float32)
    Wd2 = np.asarray(inputs["Wd2"], np.float32)
    Wv1 = np.asarray(inputs["Wv1"], np.float32)
    Wv2 = np.asarray(inputs["Wv2"], np.float32)
    w1c = np.concatenate([R * W1, (1.0 - R) * W1], axis=0)
    wv1c = np.concatenate([R * Wv1, (1.0 - R) * Wv1], axis=0)
    wall = np.concatenate([
        _pack_rows(w1c, 4), _pack_rows(W2, 2), _pack_rows(Wd1, 2),
        _pack_rows(Wd2, 4), _pack_rows(wv1c, 4), _pack_rows(Wv2, 2)],
        axis=1).astype(BF)
    assert wall.shape == (P, WCOLS)

    b2p = (np.asarray(inputs["b2"], np.float32) - LN2 * W2.sum(axis=0))
    bd2p = (np.asarray(inputs["bd2"], np.float32) - LN2 * Wd2.sum(axis=0))
    bv2p = (-LN2 * Wv2.sum(axis=0))
    ball = np.concatenate([
        np.asarray(inputs["b1"], np.float32).reshape(2, P).T,
        b2p.reshape(2, P).T,
        np.asarray(inputs["bd1"], np.float32).reshape(4, P).T,
        bd2p.reshape(4, P).T,
        bv2p.reshape(2, P).T], axis=1).astype(np.float32)
    assert ball.shape == (P, 14)

    ss = np.asarray(inputs["scalar_short"], np.float32)
    sl = np.asarray(inputs["scalar_long"], np.float32)
    vs = np.asarray(inputs["vector_short"], np.float32)
    vl = np.asarray(inputs["vector_long"], np.float32)

    in_maps = []
    for c in range(NCORES):
        m = metas[c]
        idx = np.where(m["real"], m["slot_node"], 0)

        def take2(x):
            g = (x[idx] * m["real"][:, None]).astype(np.float32)
            return _pack_rows(np.ascontiguousarray(g.T), 2).astype(BF)

        def take3(x):
            g = (x[idx] * m["real"][:, None, None]).astype(np.float32)
            t = np.ascontiguousarray(g.transpose(1, 2, 0).reshape(3 * D, M))
            return _pack_rows(t, 6).astype(BF)

        in_maps.append({
            "x_s": take2(ss), "x_l": take2(sl),
            "v_s": take3(vs), "v_l": take3(vl),
            "wall": wall, "ball": ball,
            "pm3": np.ascontiguousarray(
                m["pm3"].transpose(1, 0, 2).reshape(P, NSTRIP * 3 * P)).astype(BF),
            "tm": np.ascontiguousarray(
                m["tm"].transpose(1, 0, 2).reshape(P, NSTRIP * P)).astype(BF),
            "amat": np.ascontiguousarray(
                m["amat"].transpose(1, 0, 2).reshape(P, NSTRIP * NFC)),
            "cvec": m["cvec"],
            "ident": np.ascontiguousarray(np.eye(P, dtype=np.float32)).astype(BF),
        })
    n_valid = float((counts >= 2).sum())
    return in_maps, n_valid


def _combine(fins, n_valid):
    loss = float(sum(float(f[0]) for f in fins))
    ssq_sh = float(sum(float(f[1]) for f in fins))
    ssq_un = float(sum(float(f[2]) for f in fins))
    l2 = 0.01 * (np.sqrt(ssq_sh) + np.sqrt(ssq_un))
    if n_valid > 0:
        return np.float32(CF * (loss + n_valid * l2) / max(n_valid, 1.0))
    return np.float32(0.0)


# ============================ device program ================================

_NC_CACHE = {}


def build_nc(debug_out=False):
    key = ("nc", debug_out)
    if key in _NC_CACHE:
        return _NC_CACHE[key]
    import concourse.bass as bass
    import concourse.bacc as bacc
    import concourse.mybir as mybir
    import concourse.tile as tile

    F32 = mybir.dt.float32
    BF16 = mybir.dt.bfloat16
    AF = mybir.ActivationFunctionType
    ALU = mybir.AluOpType

    nc = bacc.Bacc("TRN2", target_bir_lowering=False, debug=False)

    d = {}
    for name, shape, dt_ in [
        ("x_s", [P, 2 * M], BF16), ("x_l", [P, 2 * M], BF16),
        ("v_s", [P, 6 * M], BF16), ("v_l", [P, 6 * M], BF16),
        ("wall", [P, WCOLS], BF16), ("ball", [P, 14], F32),
        ("pm3", [P, NSTRIP * 3 * P], BF16), ("tm", [P, NSTRIP * P], BF16),
        ("amat", [P, NSTRIP * NFC], F32), ("cvec", [NFC, NCV], F32),
        ("ident", [P, P], BF16),
    ]:
        d[name] = nc.dram_tensor(name, shape, dt_, kind="ExternalInput").ap()
    d_out = nc.dram_tensor("out", [1, 3], F32, kind="ExternalOutput").ap()
    d_inv = nc.dram_tensor("inv_scratch", [NCH * 15, P], BF16).ap()
    d_dbg = {}
    if debug_out:
        for nm, shape, dt_ in [("dbg_nsq", [P, 45], F32),
                               ("dbg_seg", [NFC, NQ], F32),
                               ("dbg_segn", [NFC, 5], F32),
                               ("dbg_irow", [1, 5 * M], F32)]:
            d_dbg[nm] = nc.dram_tensor(nm, shape, dt_, kind="ExternalOutput").ap()

    with tile.TileContext(nc) as tc, ExitStack() as ctx:
        wpool = ctx.enter_context(tc.tile_pool(name="w", bufs=1))
        feat = ctx.enter_context(tc.tile_pool(name="feat", bufs=1))
        xin = ctx.enter_context(tc.tile_pool(name="xin", bufs=2))
        small = ctx.enter_context(tc.tile_pool(name="small", bufs=1))
        rowp = ctx.enter_context(tc.tile_pool(name="rowp", bufs=2))
        junkp = ctx.enter_context(tc.tile_pool(name="junk", bufs=3))
        psp = ctx.enter_context(tc.tile_pool(name="ps", bufs=4, space="PSUM"))
        pnsq = ctx.enter_context(tc.tile_pool(name="pnsq", bufs=1, space="PSUM"))
        pseg = ctx.enter_context(tc.tile_pool(name="pseg", bufs=1, space="PSUM"))

        # ---- constants / weights (one DMA each); encoder inputs issue first
        # on the sync queue, G-phase masks go on the scalar queue ----
        def ld(name, shape, dt_, pool=wpool, eng=None):
            t = pool.tile(shape, dt_, tag=name, name=name)
            (eng or nc.sync).dma_start(out=t, in_=d[name])
            return t

        wall_sb = wpool.tile([P, WCOLS], BF16, tag="wall", name="wall")
        nc.sync.dma_start(out=wall_sb[:, 0:1536], in_=d["wall"][:, 0:1536])
        ball_sb = ld("ball", [P, 14], F32)
        # chunked feature loads: chunk 0 lands in a third of the transfer
        # time, so the encoder starts ~2x sooner than with monolithic DMAs
        xs_sb = feat.tile([P, 2 * M], BF16, tag="x_s", name="x_s")
        xl_sb = feat.tile([P, 2 * M], BF16, tag="x_l", name="x_l")
        vs_sb = feat.tile([P, 6 * M], BF16, tag="v_s", name="v_s")
        vl_sb = feat.tile([P, 6 * M], BF16, tag="v_l", name="v_l")

        def chunk_load(t, nm, kt, cc):
            csl = slice(cc * CH, (cc + 1) * CH)
            nc.sync.dma_start(
                out=t.rearrange("p (k m) -> p k m", k=kt)[:, :, csl],
                in_=d[nm].rearrange("p (k m) -> p k m", k=kt)[:, :, csl])

        for cc in range(NCH):
            chunk_load(xs_sb, "x_s", 2, cc)
            chunk_load(xl_sb, "x_l", 2, cc)
        nc.sync.dma_start(out=wall_sb[:, 1536:WCOLS], in_=d["wall"][:, 1536:WCOLS])
        for cc in range(NCH):
            chunk_load(vs_sb, "v_s", 6, cc)
            chunk_load(vl_sb, "v_l", 6, cc)
        pm3_sb = ld("pm3", [P, NSTRIP * 3 * P], BF16, eng=nc.scalar)
        tm_sb = ld("tm", [P, NSTRIP * P], BF16, eng=nc.scalar)
        amat_sb = ld("amat", [P, NSTRIP * NFC], F32, eng=nc.scalar)
        cvec_sb = ld("cvec", [NFC, NCV], F32, eng=nc.scalar)
        ident_sb = ld("ident", [P, P], BF16, eng=nc.scalar)

        def W(name, k):
            off, kt, cols = _WOFF[name]
            assert k < kt
            return wall_sb[:, off + k * cols: off + (k + 1) * cols]

        def bias(name, m_):
            offs = {"b1": 0, "b2p": 2, "bd1": 4, "bd2p": 8, "bv2p": 12}
            return ball_sb[:, offs[name] + m_: offs[name] + m_ + 1]

        ones_col = wpool.tile([P, 1], BF16, tag="ones_col", name="ones_col")
        nc.vector.memset(ones_col, 1.0)
        ones32 = wpool.tile([NFC, 1], F32, tag="ones32", name="ones32")
        nc.vector.memset(ones32, 1.0)
        eps_sb = wpool.tile([P, 1], F32, tag="eps", name="eps")
        nc.vector.memset(eps_sb, 1e-20)

        # ---- persistent per-chunk feature tiles ----
        # sets order: 0=sh 1=un 2=v0 3=v1 4=v2 ; each set has k-tiles 0,1
        u_t = [[[feat.tile([P, CH], BF16, tag=f"u{s}_{k}_{c}", name=f"u{s}_{k}_{c}")
                 for c in range(NCH)] for k in range(2)] for s in range(5)]
        n_t = [[[feat.tile([P, CH], BF16, tag=f"n{s}_{k}_{c}", name=f"n{s}_{k}_{c}")
                 for c in range(NCH)] for k in range(2)] for s in range(5)]

        nsq_ps = pnsq.tile([P, 45], F32, tag="nsq", name="nsq_ps")

        def mm_chunk(pt, w_name, rhs_list, m_):
            kt = len(rhs_list)
            for k in range(kt):
                nc.tensor.matmul(pt[:, 0:CH], W(w_name, k)[:, m_ * P:(m_ + 1) * P],
                                 rhs_list[k], start=(k == 0), stop=(k == kt - 1))

        def xsl(t, k, c):
            return t[:, k * M + c * CH: k * M + (c + 1) * CH]

        # ---- encoder phase A: h1 = softplus(x@W1c + b1), split Exp/Ln ----
        e_h1, t_h1, s_all = [], [], []
        for c in range(NCH):
            ew = xin.tile([P, 2 * CH], BF16, tag="eh1", name=f"eh1_{c}", bufs=2)
            for m_ in range(2):
                pt = psp.tile([P, 512], F32, tag="ps", name="ps")
                mm_chunk(pt, "w1c", [xsl(xs_sb, 0, c), xsl(xs_sb, 1, c),
                                     xsl(xl_sb, 0, c), xsl(xl_sb, 1, c)], m_)
                nc.scalar.activation(ew[:, m_ * CH:(m_ + 1) * CH], pt[:, 0:CH],
                                     AF.Exp, bias=bias("b1", m_))
            e_h1.append(ew)
        for c in range(NCH):
            tw = xin.tile([P, 2 * CH], BF16, tag="th1", name=f"th1_{c}", bufs=2)
            nc.scalar.activation(tw, e_h1[c], AF.Ln, bias=1.0)
            t_h1.append(tw)
        for c in range(NCH):
            s_ = []
            for m_ in range(2):
                pt = psp.tile([P, 512], F32, tag="ps", name="ps")
                mm_chunk(pt, "w2", [t_h1[c][:, 0:CH], t_h1[c][:, CH:2 * CH]], m_)
                t = xin.tile([P, CH], BF16, tag=f"s{m_}", name=f"s{m_}_{c}", bufs=2)
                nc.vector.tensor_scalar_add(t, pt[:, 0:CH], bias("b2p", m_))
                s_.append(t)
            s_all.append(s_)

        # ---- encoder phase B: hd + v1 exps, wide Ln, then dec/v2 evacs ----
        e_B, t_B = [], []
        for c in range(NCH):
            ew = xin.tile([P, 10 * CH], BF16, tag="eB", name=f"eB_{c}", bufs=2)
            for m_ in range(4):
                pt = psp.tile([P, 512], F32, tag="ps", name="ps")
                mm_chunk(pt, "wd1", s_all[c], m_)
                nc.scalar.activation(ew[:, m_ * CH:(m_ + 1) * CH], pt[:, 0:CH],
                                     AF.Exp, bias=bias("bd1", m_))
            for dd in range(3):
                rhs = [xsl(vs_sb, 2 * dd, c), xsl(vs_sb, 2 * dd + 1, c),
                       xsl(vl_sb, 2 * dd, c), xsl(vl_sb, 2 * dd + 1, c)]
                for m_ in range(2):
                    pt = psp.tile([P, 512], F32, tag="ps", name="ps")
                    mm_chunk(pt, "wv1c", rhs, m_)
                    j = 4 + 2 * dd + m_
                    nc.scalar.activation(ew[:, j * CH:(j + 1) * CH], pt[:, 0:CH],
                                         AF.Exp)
            e_B.append(ew)
        for c in range(NCH):
            tw = xin.tile([P, 10 * CH], BF16, tag="tB", name=f"tB_{c}", bufs=2)
            nc.scalar.activation(tw, e_B[c], AF.Ln, bias=1.0)
            t_B.append(tw)

        xsq_t = [[None] * 2 for _ in range(5)]

        def sq_and_nsq(s, k, c, src):
            # xsq = src^2; the one-column matmuls that reduce it to nsq are
            # emitted after the chunk's evacs so each column's two-matmul
            # accumulation group is contiguous
            t = feat.tile([P, CH], BF16, tag=f"xsq{s}_{k}", name=f"xsq{s}_{k}")
            if s < 2:
                nc.gpsimd.tensor_mul(t, src, src)
            else:
                nc.vector.tensor_mul(t, src, src)
            xsq_t[s][k] = t

        def nsq_mms(c):
            # column layout is (chunk, set, local-strip)-major so each chunk's
            # 15 columns are contiguous for the transpose
            for s in range(5):
                for ls in range(3):
                    col = c * 15 + s * 3 + ls
                    for k in range(2):
                        nc.tensor.matmul(
                            nsq_ps[:, col:col + 1],
                            xsq_t[s][k][:, ls * P:(ls + 1) * P], ones_col,
                            start=(k == 0), stop=(k == 1))

        def msum(out_t, in0, in1, acc):
            nc.vector.scalar_tensor_tensor(
                out=out_t, in0=in0, scalar=1.0, in1=in1,
                op0=ALU.bypass, op1=ALU.mult, accum_out=acc)

        Q_b = [small.tile([P, NQ], F32, tag=f"Q{b}", name=f"Q{b}")
               for b in range(NSTRIP)]
        d_inv_rd = d_inv.rearrange("(o c r) p -> c o (r p)", o=1, c=NCH, r=15)

        def strip_work(b):
            cb, ls = b // 3, (b % 3)
            lsl = slice(ls * P, (ls + 1) * P)
            pmb = pm3_sb[:, b * 3 * P: b * 3 * P + P]
            pm3b = pm3_sb[:, b * 3 * P: (b + 1) * 3 * P]
            tmb = tm_sb[:, b * P:(b + 1) * P]
            Q = Q_b[b]

            def gpair(pt, col0, tiles):
                for k in range(2):
                    nc.tensor.matmul(pt[:, col0:col0 + P], tiles[k][cb][:, lsl],
                                     tiles[k][cb][:, lsl],
                                     start=(k == 0), stop=(k == 1))

            # unnormalized grams (sh + 3 v dims) packed in one bank
            gu = psp.tile([P, 512], F32, tag="ps", name="gu")
            gpair(gu, 0, u_t[0])
            for dd in range(3):
                gpair(gu, P + dd * P, u_t[2 + dd])
            j5 = junkp.tile([P, P], BF16, tag="jk1", name="j5")
            msum(j5, gu[:, 0:P], pmb, Q[:, QSOFF:QSOFF + 1])
            j6 = junkp.tile([P, 3 * P], BF16, tag="jk3", name="j6")
            msum(j6, gu[:, P:4 * P], pm3b, Q[:, QVOFF:QVOFF + 1])

            # normalized grams: [sh | un] bank and [v0 | v1 | v2] bank
            gn = psp.tile([P, 512], F32, tag="ps", name="gn")
            gpair(gn, 0, n_t[0])
            gpair(gn, P, n_t[1])
            gv = psp.tile([P, 512], F32, tag="ps", name="gv")
            for dd in range(3):
                gpair(gv, dd * P, n_t[2 + dd])

            spm = junkp.tile([P, P], BF16, tag="spm", name="spm", bufs=2)
            msum(spm, gn[:, 0:P], pmb, Q[:, QB_SH:QB_SH + 1])
            nc.scalar.activation(junkp.tile([P, P], BF16, tag="jk1", name="j1"),
                                 spm, AF.Square, accum_out=Q[:, QA_SH:QA_SH + 1])
            upm = junkp.tile([P, P], BF16, tag="upm", name="upm", bufs=2)
            msum(upm, gn[:, P:2 * P], pmb, Q[:, QJK:QJK + 1])
            nc.scalar.activation(junkp.tile([P, P], BF16, tag="jk1", name="j2"),
                                 upm, AF.Square, accum_out=Q[:, QA_UN:QA_UN + 1])
            j3 = junkp.tile([P, P], BF16, tag="jk1", name="j3")
            msum(j3, gn[:, P:2 * P], tmb, Q[:, QC_UN:QC_UN + 1])

            dpm = junkp.tile([P, 3 * P], BF16, tag="dpm", name="dpm", bufs=2)
            msum(dpm, gv[:, 0:3 * P], pm3b, Q[:, QB_DIR:QB_DIR + 1])
            nc.scalar.activation(junkp.tile([P, 3 * P], BF16, tag="jk3", name="j4"),
                                 dpm, AF.Square, accum_out=Q[:, QA_DIR:QA_DIR + 1])

        # dec/v2 evacs (DVE) + squared features + per-strip nsq columns
        for c in range(NCH):
            hd = [t_B[c][:, m_ * CH:(m_ + 1) * CH] for m_ in range(4)]
            for m_ in range(4):
                pt = psp.tile([P, 512], F32, tag="ps", name="ps")
                mm_chunk(pt, "wd2", hd, m_)
                s, k = (0, m_) if m_ < 2 else (1, m_ - 2)
                nc.vector.tensor_scalar_add(u_t[s][k][c], pt[:, 0:CH],
                                            bias("bd2p", m_))
                sq_and_nsq(s, k, c, u_t[s][k][c])
            for dd in range(3):
                v1 = [t_B[c][:, (4 + 2 * dd + k) * CH:(5 + 2 * dd + k) * CH]
                      for k in range(2)]
                for m_ in range(2):
                    pt = psp.tile([P, 512], F32, tag="ps", name="ps")
                    mm_chunk(pt, "wv2", v1, m_)
                    nc.vector.tensor_scalar_add(u_t[2 + dd][m_][c], pt[:, 0:CH],
                                                bias("bv2p", m_))
                    sq_and_nsq(2 + dd, m_, c, u_t[2 + dd][m_][c])
            nsq_mms(c)

        # ---- inverse norms: inv = (nsq+eps)^(-1/2); transpose on the tensor
        # engine to node-row layout so the DRAM bounce is contiguous ----
        nsq_sb = small.tile([P, 45], F32, tag="nsq_sb", name="nsq_sb")
        nc.vector.tensor_copy(nsq_sb, nsq_ps)
        lnn = small.tile([P, 45], F32, tag="lnn", name="lnn")
        nc.scalar.activation(lnn, nsq_ps, AF.Ln, bias=eps_sb[:, 0:1])
        invN = small.tile([P, 45], BF16, tag="invN", name="invN")
        nc.scalar.activation(invN, lnn, AF.Exp, scale=-0.5)
        for c in range(NCH):
            invc = invN[:, c * 15:(c + 1) * 15]
            invT_ps = pseg.tile([15, P], BF16, tag="invT", name="invT")
            nc.tensor.transpose(invT_ps, invc, ident_sb)
            invT_c = junkp.tile([15, P], BF16, tag="invT_sb", name=f"invT{c}",
                                bufs=2)
            nc.vector.tensor_copy(invT_c, invT_ps)
            nc.sync.dma_start(out=d_inv[c * 15:(c + 1) * 15, :], in_=invT_c)

        # normalize + per-strip grams/reductions, chunk by chunk
        for c in range(NCH):
            irow_c = rowp.tile([1, 15 * P], BF16, tag="irow", name=f"irow{c}",
                               bufs=2)
            nc.sync.dma_start(out=irow_c, in_=d_inv_rd[c])
            for s in range(5):
                B_sc = rowp.tile([P, CH], BF16, tag="Bb", name=f"B{s}_{c}", bufs=3)
                nc.gpsimd.partition_broadcast(
                    B_sc, irow_c[0:1, s * CH:(s + 1) * CH])
                for k in range(2):
                    nc.vector.tensor_mul(n_t[s][k][c], u_t[s][k][c], B_sc)
        for b in range(NSTRIP):
            strip_work(b)

        # ---- segment reduction + final combine ----
        segq_ps = pseg.tile([NFC, NQ + 5], F32, tag="seg", name="seg")
        seg_ps = segq_ps[:, 0:NQ]
        segn_ps = segq_ps[:, NQ:NQ + 5]
        nsq_v = nsq_sb.rearrange("p (c s b) -> p c s b", c=NCH, s=5)
        for b in range(NSTRIP):
            nc.tensor.matmul(seg_ps, amat_sb[:, b * NFC:(b + 1) * NFC], Q_b[b],
                             start=(b == 0), stop=(b == NSTRIP - 1))
        for b in range(NSTRIP):
            nc.tensor.matmul(segn_ps, amat_sb[:, b * NFC:(b + 1) * NFC],
                             nsq_v[:, b // 3, :, b % 3],
                             start=(b == 0), stop=(b == NSTRIP - 1))
        segq = small.tile([NFC, NQ + 5], F32, tag="segs", name="segs")
        nc.vector.tensor_copy(segq, segq_ps)
        segs = segq[:, 0:NQ]
        segn = segq[:, NQ:NQ + 5]

        acc = small.tile([NFC, 3], F32, tag="acc", name="acc")
        junkq = small.tile([NFC, NQ + 5], F32, tag="junkq", name="junkq")
        # acc0 = sum_q cvec[q]*segq[q] over Q cols + segn cols, then + const
        nc.vector.scalar_tensor_tensor(
            out=junkq, in0=segq, scalar=1.0, in1=cvec_sb[:, 0:NQ + 5],
            op0=ALU.bypass, op1=ALU.mult, accum_out=acc[:, 0:1])
        nc.vector.tensor_add(acc[:, 0:1], acc[:, 0:1], cvec_sb[:, 14:15])
        nc.vector.tensor_copy(acc[:, 1:2], segn[:, 0:1])
        nc.vector.tensor_copy(acc[:, 2:3], segn[:, 1:2])

        if debug_out:
            nc.sync.dma_start(out=d_dbg["dbg_nsq"], in_=nsq_sb)
            nc.sync.dma_start(out=d_dbg["dbg_seg"], in_=segs)
            nc.sync.dma_start(out=d_dbg["dbg_segn"], in_=segn)
            irow_f = rowp.tile([1, 5 * M], F32, tag="irow_f", name="irow_f", bufs=1)
            nc.vector.tensor_copy(irow_f, irow)
            nc.sync.dma_start(out=d_dbg["dbg_irow"], in_=irow_f)

        fin_ps = pseg.tile([1, 3], F32, tag="fin", name="fin")
        nc.tensor.matmul(fin_ps, ones32, acc, start=True, stop=True)
        fin_sb = small.tile([1, 3], F32, tag="fin_sb", name="fin_sb")
        nc.vector.tensor_copy(fin_sb, fin_ps)
        nc.sync.dma_start(out=d_out, in_=fin_sb)

    nc.compile()
    _NC_CACHE[key] = nc
    return nc


# ============================== entry point =================================

def kernel(**inputs) -> np.ndarray:
    from concourse.bass_utils import run_bass_kernel_spmd

    in_maps, n_valid = _shard_inputs(inputs)
    nc = build_nc()
    res = run_bass_kernel_spmd(nc, in_maps, core_ids=list(range(NCORES)))
    fins = [r["out"].reshape(3) for r in res.results]
    return _combine(fins, n_valid)
